# revision 48
# baseline (speedup 1.0000x reference)
"""Causal attention block (LN -> QKV -> causal MHA -> out-proj) on 8 trn2
NeuronCores via Bass/Tile.

Sharding: core c handles batch b=c//2 and head-group g=c%2 (8 of 16 heads).
Data parallel over batch, tensor parallel over heads; the out-proj partial
sums (2 per batch) are reduced on the host during the gather, so the device
program needs no collectives and is pure SPMD.

v2 design (vs 613us baseline):
  - All big matmuls run on bf16 operands (fp32 PSUM accumulation): halves
    SBUF footprint, DMA bytes and LDWEIGHTS traffic.
  - The PE is kept continuously busy so the HAM activity monitor never
    re-throttles the 2.4 GHz clock: LN stats matmuls overlap the x DMA;
    the rstd-independent QKV main matmuls (x is kept UNSCALED) cover the
    LN postprocessing; rstd is folded into the PSUM->SBUF evacuation of
    Q^T/K^T (scalar_tensor_tensor with a PE-broadcast rstd tile) and into
    V's ACT-engine copy (per-partition scale operand).
  - Causal diagonal trimming for query tiles t>=1: the 4 diagonal key
    chunks only compute/apply exp+mask+PV on columns [m*128:512]; masking
    shrinks to one [128,128] triangle multiply per (chunk, head).
  - Softmax denominators (the free 65th ones-column of V) are inverted
    with reciprocal_approx_fast (~0.9us) instead of the 8-cycle/element
    DVE reciprocal (3.6us each / 17us for the LN row).
  - o^T stays in SBUF (no DRAM scratch roundtrip); PSUM accumulator banks
    are evacuated immediately after the last PV so the normalization chain
    never blocks the next tile; normalization output is DMA'd SBUF->SBUF
    into the per-pair o^T tile (the DMA performs the partition shift).
  - The out-projection is interleaved with pair 3's attention tiles and
    reads o^T straight from SBUF.
"""

import numpy as np

import concourse.bass as bass
import concourse.mybir as mybir
import concourse.tile as tile_mod

# ----------------------------------------------------------------------------
# Workaround for this walrus build rejecting instructions that carry more than
# MAX_WAITS semaphore waits ("Too many sync wait commands" in CoreV3GenImpl
# setupSyncWait). Split excess waits onto single-wait NOP carrier instructions
# emitted just before the original instruction on the same engine: program
# order on the sequencer makes this semantically identical.
# ----------------------------------------------------------------------------
_MAX_WAITS = 1
_orig_add_instruction = tile_mod.TileContext._add_instruction
_carrier_id = [0]


def _split_waits_add_instruction(self, inst):
    si = inst.sync_info
    if (
        si is not None
        and si.on_wait
        and len(si.on_wait) > _MAX_WAITS
        and inst.engine != mybir.EngineType.Unassigned
    ):
        waits = list(si.on_wait)
        keep = waits[-_MAX_WAITS:]
        for w in waits[:-_MAX_WAITS]:
            _carrier_id[0] += 1
            nop = mybir.InstNoOp(name=f"I-waitc-{_carrier_id[0]}")
            nop.engine = inst.engine
            nop.sync_info = mybir.SyncInfo(on_wait=[w], on_update=[])
            _orig_add_instruction(self, nop)
        inst.sync_info = mybir.SyncInfo(
            on_wait=keep,
            on_update=list(si.on_update) if si.on_update else [],
        )
    _orig_add_instruction(self, inst)


tile_mod.TileContext._add_instruction = _split_waits_add_instruction

from concourse.vector_clock import ScopedClock


def _patched_drain_and_barrier(self, tick_clock, wait_clock):
    nc = self.nc
    carrier = nc.sync.nop(nofuse=True)
    wait_clock.add_sem_waits(carrier.ins, ScopedClock({None: tick_clock.global_clock}))
    si = carrier.ins.sync_info
    waits = list(si.on_wait) if si is not None and si.on_wait else []
    if len(waits) > _MAX_WAITS:
        carrier.ins.sync_info = mybir.SyncInfo(
            on_wait=waits[:_MAX_WAITS],
            on_update=list(si.on_update) if si.on_update else [],
        )
        rest = waits[_MAX_WAITS:]
        while rest:
            extra = nc.sync.nop(nofuse=True)
            extra.ins.sync_info = mybir.SyncInfo(
                on_wait=rest[:_MAX_WAITS], on_update=[])
            rest = rest[_MAX_WAITS:]

    nc.sync.drain()
    nc.all_engine_barrier()
    assert self.sems is not None
    popped = nc._tile_sem_poison_stack.pop()
    assert popped is self._sem_poison
    nc.clear_and_free_semaphores(list(self.sems.allocated().values()))
    nc.all_engine_barrier()


tile_mod.TileContext._drain_and_barrier = _patched_drain_and_barrier

# ----------------------------------------------------------------------------

F32 = mybir.dt.float32
F32R = mybir.dt.float32r
BF16 = mybir.dt.bfloat16
ALU = mybir.AluOpType
ACT_EXP = mybir.ActivationFunctionType.Exp
ACT_LN = mybir.ActivationFunctionType.Ln

U32 = mybir.dt.uint32
ONE_BITS = int(np.float32(1.0).view(np.uint32))

B = 4
TOK = 2048
DIM = 1024
HEADS = 16
DH = 64
HEADS_PC = 8          # heads per core
INNER_PC = HEADS_PC * DH  # 512
NPAIRS = HEADS_PC // 2
QT = 512              # query tile
KC = 128              # key-token chunk
NKD = DIM // 128      # 8 contraction chunks
NQT = TOK // QT       # 4 query tiles
NTT = TOK // KC       # 16 token chunks
EPS = 1e-5
SCALE = DH ** -0.5


def _r(ap):
    """View an fp32 AP as float32r for full-rate PE matmuls."""
    return ap.bitcast(F32R)


def build_program():
    nc = bass.Bass()
    xT = nc.declare_dram_parameter("xT", [DIM, TOK], BF16, isOutput=False)
    w = nc.declare_dram_parameter("w", [DIM, 3 * INNER_PC], BF16, isOutput=False)
    seed = nc.declare_dram_parameter("seed", [2, 3 * INNER_PC], F32R, isOutput=False)
    wo = nc.declare_dram_parameter("wo", [INNER_PC, DIM], BF16, isOutput=False)
    masks = nc.declare_dram_parameter("masks", [4, KC, QT], BF16, isOutput=False)
    out = nc.declare_dram_parameter("out", [TOK, DIM], F32, isOutput=True)
    rstd_dram = nc.dram_tensor("rstd_scr", [1, TOK], F32R)

    w4 = w.rearrange("(kc p) (g c) -> p kc g c", p=128, c=128)
    wo4 = wo.rearrange("(pp p) d -> p pp d", p=128)

    with tile_mod.TileContext(nc) as tc, nc.allow_low_precision(
            "bf16 matmul operands; all accumulation in fp32 PSUM"):
        with (
            tc.tile_pool(name="const", bufs=1) as const,
            tc.tile_pool(name="xtp", bufs=NKD) as xtp,
            tc.tile_pool(name="wqkp", bufs=NPAIRS) as wqkp,
            tc.tile_pool(name="wbig", bufs=1) as wbig,
            tc.tile_pool(name="vpool", bufs=NTT) as vpool,
            tc.tile_pool(name="qkp", bufs=2) as qkp,
            tc.tile_pool(name="opp", bufs=NPAIRS) as opp,
            tc.tile_pool(name="psb", bufs=3) as ppool,
            tc.tile_pool(name="orawp", bufs=4) as orawp,
            tc.tile_pool(name="osbp", bufs=4) as osbp,
            tc.tile_pool(name="sqp", bufs=2) as sqp,
            tc.tile_pool(name="recp", bufs=4) as recp,
            tc.tile_pool(name="outsb", bufs=2) as outsbp,
        ):
            # ---------------- constants / weights ----------------
            ones_col = const.tile([128, 1], BF16, tag="ones_col")
            nc.vector.memset(ones_col, 1.0)
            ones_row = const.tile([1, 128], F32R, tag="ones_row")
            nc.vector.memset(ones_row.bitcast(U32), ONE_BITS)
            ones_row_bf = const.tile([1, 128], BF16, tag="ones_row_bf")
            nc.vector.memset(ones_row_bf, 1.0)
            eps_t = const.tile([1, 1], F32, tag="eps")
            nc.vector.memset(eps_t, EPS)
            seed_sb = const.tile([2, 3 * INNER_PC], F32R, tag="seed")
            nc.sync.dma_start(out=seed_sb, in_=seed[:, :])
            mask_sb = const.tile([KC, 4, QT], BF16, tag="mask")
            for m in range(4):
                nc.scalar.dma_start(out=mask_sb[:, m, :], in_=masks[m, :, :])
            onmr2 = const.tile([2, TOK], F32R, tag="onmr2")  # row0=std, row1=-mu
            r0 = const.tile([1, TOK], F32R, tag="r0")  # sums -> -mu
            r1 = const.tile([1, TOK], F32R, tag="r1")  # sumsq -> ex2 -> var -> L
            rstd_row = const.tile([1, TOK], F32R, tag="rstd_row")  # musq -> rstd
            bc_sb = const.tile([128, TOK], F32, tag="bc")    # rstd col-broadcast
            rstd_col = const.tile([128, NTT], F32R, tag="rstd_col")

            # ---------------- phase A: x load + LN stats ----------------
            xt = []
            with tc.tile_pool(name="pstat", bufs=1, space="PSUM") as pstat:
                sum_ps = pstat.tile([1, TOK], F32, tag="sum")
                sq_ps = pstat.tile([1, TOK], F32, tag="sq")
                for kc in range(NKD):
                    t_ = xtp.tile([128, TOK], BF16, tag="xt", name=f"xt{kc}")
                    nc.sync.dma_start(out=t_, in_=xT[kc * 128:(kc + 1) * 128, :])
                    xt.append(t_)
                    sq_t = sqp.tile([128, TOK], BF16, tag="sq_t")
                    nc.vector.tensor_mul(sq_t, t_, t_)
                    for s in range(NQT):
                        sl = slice(s * QT, (s + 1) * QT)
                        nc.tensor.matmul(
                            out=sum_ps[0:1, sl], lhsT=ones_col, rhs=t_[:, sl],
                            start=(kc == 0), stop=(kc == NKD - 1))
                        nc.tensor.matmul(
                            out=sq_ps[0:1, sl], lhsT=ones_col, rhs=sq_t[:, sl],
                            start=(kc == 0), stop=(kc == NKD - 1))
                # evacuate stats (ACT + DVE in parallel) so pstat banks free
                nc.scalar.copy(r0, sum_ps)
                nc.vector.tensor_copy(r1, sq_ps)

            wqk = []
            for p in range(NPAIRS):
                t_ = wqkp.tile([128, NKD, 256], BF16, tag="wqk", name=f"wqk{p}")
                eng = nc.scalar if p == 0 else nc.sync
                for kc in range(NKD):
                    eng.dma_start(out=t_[:, kc, 0:128], in_=w4[:, kc, p, :])
                    eng.dma_start(out=t_[:, kc, 128:256],
                                  in_=w4[:, kc, 4 + p, :])
                wqk.append(t_)
            wv = wbig.tile([128, NKD, INNER_PC], BF16, tag="wv")
            for kc in range(NKD):
                nc.scalar.dma_start(
                    out=wv[:, kc, :].rearrange("p (g c) -> p g c", c=128),
                    in_=w4[:, kc, 8:12, :])
            wo_sb = wbig.tile([128, NPAIRS, DIM], BF16, tag="wo")
            for p in range(NPAIRS):
                nc.sync.dma_start(out=wo_sb[:, p, :], in_=wo4[:, p, :])


            with (
                tc.tile_pool(name="pss", bufs=2, space="PSUM") as pss,
                tc.tile_pool(name="pso", bufs=2, space="PSUM") as pso,
                tc.tile_pool(name="scr", bufs=2, space="PSUM") as scr,
            ):
                # ------------- LN postprocessing (DVE/ACT only) -------------
                nc.vector.tensor_scalar_mul(r0, r0, -1.0 / DIM)          # -mu
                nc.scalar.dma_start(out=onmr2[1:2, :], in_=r0)           # -mu row
                nc.vector.tensor_scalar_mul(r1, r1, 1.0 / DIM)           # ex2
                nc.scalar.square(rstd_row, r0)                           # mu^2
                nc.vector.tensor_sub(r1, r1, rstd_row)                   # var
                nc.scalar.activation(out=r1, in_=r1,
                                     func=ACT_LN, bias=eps_t, scale=1.0)
                nc.scalar.activation(out=rstd_row, in_=r1,
                                     func=ACT_EXP, scale=-0.5)           # rstd
                nc.scalar.activation(out=onmr2[0:1, :], in_=r1,
                                     func=ACT_EXP, scale=0.5)            # std
                # rstd transposed to columns via a DRAM roundtrip (for V's
                # per-partition ACT scale)
                nc.scalar.dma_start(out=rstd_dram[:, :], in_=rstd_row)
                nc.scalar.dma_start(
                    out=rstd_col,
                    in_=rstd_dram.rearrange("o (i p) -> p (o i)", p=128))

                qT = {}
                kT = {}

                def emit_qk(p, dst, t, pool, defer_stt=False):
                    """Project 128 cols of Q^T (dst=0) or K^T (dst=1) for
                    tokens of tile t; fold rstd via STT on evacuation."""
                    if dst == 0 and p not in qT:
                        qT[p] = qkp.tile([128, TOK], BF16, tag="qT",
                                         name=f"qT{p}")
                    if dst == 1 and p not in kT:
                        kT[p] = qkp.tile([128, TOK], BF16, tag="kT",
                                         name=f"kT{p}")
                    dstT = qT[p] if dst == 0 else kT[p]
                    sl = slice(t * QT, (t + 1) * QT)
                    tag = "s_ps" if pool is pss else ("o_ps" if pool is pso else "scr")
                    ps = pool.tile([128, QT], F32, tag=tag, name=f"qk{p}{dst}{t}")
                    for kc in range(NKD):
                        nc.tensor.matmul(
                            out=ps, lhsT=wqk[p][:, kc, dst * 128:dst * 128 + 128],
                            rhs=xt[kc][:, sl], start=(kc == 0), stop=False)
                    nc.tensor.matmul(
                        out=ps,
                        lhsT=seed_sb[:, dst * INNER_PC + p * 128:
                                     dst * INNER_PC + (p + 1) * 128],
                        rhs=onmr2[:, sl], start=False, stop=True)

                    def stt():
                        nc.vector.scalar_tensor_tensor(
                            out=dstT[:, sl], in0=ps, scalar=1.0,
                            in1=bc_sb[:, sl], op0=ALU.mult, op1=ALU.mult)
                    if defer_stt:
                        return stt
                    stt()

                v_sb = {}

                def emit_v(tt, pool):
                    """V (token-major) for token chunk tt + ones column."""
                    tsl = slice(tt * KC, (tt + 1) * KC)
                    tag = "s_ps" if pool is pss else ("o_ps" if pool is pso else "scr")
                    ps = pool.tile([128, INNER_PC], F32, tag=tag, name=f"v{tt}")
                    for kc in range(NKD):
                        nc.tensor.matmul(
                            out=ps, lhsT=xt[kc][:, tsl], rhs=wv[:, kc, :],
                            start=(kc == 0), stop=False)
                    nc.tensor.matmul(
                        out=ps, lhsT=onmr2[:, tsl],
                        rhs=seed_sb[:, 2 * INNER_PC:3 * INNER_PC],
                        start=False, stop=True)
                    vt = vpool.tile([128, HEADS_PC, DH + 1], BF16, tag="v_sb",
                                    name=f"vsb{tt}")
                    nc.scalar.mul(vt[:, :, 0:DH],
                                  ps.rearrange("p (h d) -> p h d", d=DH),
                                  rstd_col.bitcast(F32)[:, tt:tt + 1])
                    nc.vector.memset(vt[:, :, DH:DH + 1], 1.0)
                    v_sb[tt] = vt

                # prologue PE work: QK(0) tile0 + V0/V1 in pss/pso slots,
                # then rstd transpose + V2/V3 + bc in scratch.
                stt_q = emit_qk(0, 0, 0, pss, defer_stt=True)
                stt_k = emit_qk(0, 1, 0, pss, defer_stt=True)
                emit_v(0, pso)
                emit_v(1, pso)
                emit_v(2, scr)
                emit_v(3, scr)
                for s in range(NQT):
                    sl = slice(s * QT, (s + 1) * QT)
                    bc_ps = scr.tile([128, QT], F32, tag="scr", name=f"bc{s}")
                    nc.tensor.matmul(out=bc_ps, lhsT=ones_row,
                                     rhs=rstd_row[:, sl], start=True, stop=True)
                    nc.vector.tensor_copy(bc_sb[:, sl], bc_ps)
                stt_q()
                stt_k()

                O_p = [opp.tile([128, TOK], BF16, tag="O_p", name=f"Op{p}")
                       for p in range(NPAIRS)]

                def emit_outproj(t):
                    for tt in range(4 * t, 4 * t + 4):
                        tsl = slice(tt * KC, (tt + 1) * KC)
                        for nb in range(DIM // QT):
                            nsl = slice(nb * QT, (nb + 1) * QT)
                            ps = scr.tile([128, QT], F32, tag="scr",
                                          name=f"out{tt}{nb}")
                            for p_ in range(NPAIRS):
                                nc.tensor.matmul(
                                    out=ps, lhsT=O_p[p_][:, tsl],
                                    rhs=wo_sb[:, p_, nsl],
                                    start=(p_ == 0), stop=(p_ == NPAIRS - 1))
                            ob = outsbp.tile([128, QT], F32, tag="out_sb")
                            nc.scalar.copy(ob, ps)
                            nc.sync.dma_start(out=out[tsl, nsl], in_=ob)

                # ---------------- attention pair loop ----------------
                pending = [None]

                def att_tile(p, t):
                    q_, k_ = qT[p], kT[p]
                    if t == 0:
                        cl = [(c, 0) for c in range(4)]
                    else:
                        cl = [(4 * t + m, m * 128) for m in range(4)] + \
                             [(c, 0) for c in range(4 * t)]
                    n = len(cl)
                    o_ps = [pso.tile([DH + 1, QT], F32, tag="o_ps",
                                     name=f"o{p}{t}{h}") for h in range(2)]
                    s_tiles = {}

                    def emit_S(i):
                        kt, lo = cl[i]
                        st = pss.tile([128, 2 * QT], F32, tag="s_ps",
                                      name=f"s{p}{t}{i}")
                        csl = slice(kt * KC, (kt + 1) * KC)
                        qsl = slice(t * QT + lo, (t + 1) * QT)
                        for h in range(2):
                            nc.tensor.matmul(
                                out=st[:, h * QT + lo:(h + 1) * QT],
                                lhsT=k_[h * DH:(h + 1) * DH, csl],
                                rhs=q_[h * DH:(h + 1) * DH, qsl],
                                start=True, stop=True)
                        s_tiles[i] = st

                    emit_S(0)
                    if n > 1:
                        emit_S(1)
                    if pending[0] is not None:
                        pending[0]()
                        pending[0] = None
                    for i, (kt, lo) in enumerate(cl):
                        st = s_tiles.pop(i)
                        pt = ppool.tile([128, 2 * QT], BF16, tag="p_sb",
                                        name=f"pt{p}{t}{i}")
                        if lo == 0:
                            nc.scalar.activation(out=pt, in_=st,
                                                 func=ACT_EXP, scale=SCALE)
                        else:
                            for h in range(2):
                                hsl = slice(h * QT + lo, (h + 1) * QT)
                                nc.scalar.activation(out=pt[:, hsl],
                                                     in_=st[:, hsl],
                                                     func=ACT_EXP, scale=SCALE)
                        if t == 0:
                            for h in range(2):
                                nc.vector.tensor_mul(
                                    pt[:, h * QT:(h + 1) * QT],
                                    pt[:, h * QT:(h + 1) * QT],
                                    mask_sb[:, i, :])
                        elif i < 4:
                            m = i
                            for h in range(2):
                                bsl = slice(h * QT + m * 128,
                                            h * QT + (m + 1) * 128)
                                nc.vector.tensor_mul(
                                    pt[:, bsl], pt[:, bsl],
                                    mask_sb[:, 0, 0:128])
                        for h in range(2):
                            nc.tensor.matmul(
                                out=o_ps[h][:, lo:QT],
                                lhsT=v_sb[kt][:, 2 * p + h, :],
                                rhs=pt[:, h * QT + lo:(h + 1) * QT],
                                start=(i == 0), stop=(i == n - 1))
                        # PE filler hooks: next QK tiles / V prefetch / outproj
                        if i == 0:
                            if p == 0 and t < 3:
                                emit_v(4 * (t + 1), scr)
                                emit_v(4 * (t + 1) + 1, scr)
                            if t < 3:
                                emit_qk(p, 0, t + 1, scr)
                            elif p < 3:
                                emit_qk(p + 1, 0, 0, scr)
                        elif i == 1:
                            if p == 0 and t < 3:
                                emit_v(4 * (t + 1) + 2, scr)
                                emit_v(4 * (t + 1) + 3, scr)
                            if t < 3:
                                emit_qk(p, 1, t + 1, scr)
                            elif p < 3:
                                emit_qk(p + 1, 1, 0, scr)
                        elif i == 2 and p == 3 and t >= 1:
                            emit_outproj(t - 1)
                        if i + 2 < n:
                            emit_S(i + 2)

                    # tile end: invert denominators + evacuate PSUM fast
                    rec = [recp.tile([1, QT], F32R, tag="rec",
                                     name=f"rec{p}{t}{h}") for h in range(2)]
                    den = [recp.tile([1, QT], F32, tag="den",
                                     name=f"den{p}{t}{h}") for h in range(2)]
                    oraw = []
                    for h in range(2):
                        nc.vector.tensor_copy(den[h], o_ps[h][DH:DH + 1, :])
                        orw = orawp.tile([DH, QT], BF16, tag="oraw",
                                         name=f"oraw{p}{t}{h}")
                        nc.vector.tensor_copy(orw, o_ps[h][0:DH, :])
                        oraw.append(orw)
                    for h in range(2):
                        nc.vector.reciprocal(rec[h], den[h])

                    def pend(p=p, t=t, rec=rec, oraw=oraw):
                        qsl = slice(t * QT, (t + 1) * QT)
                        for h in range(2):
                            rb = scr.tile([DH, QT], F32, tag="scr",
                                          name=f"rb{p}{t}{h}")
                            nc.tensor.matmul(
                                out=rb, lhsT=ones_row[:, 0:DH],
                                rhs=rec[h], start=True, stop=True)
                            ob = osbp.tile([DH, QT], BF16, tag="osb",
                                           name=f"osb{p}{t}{h}")
                            nc.vector.scalar_tensor_tensor(
                                out=ob, in0=oraw[h], scalar=1.0, in1=rb,
                                op0=ALU.mult, op1=ALU.mult)
                            nc.sync.dma_start(
                                out=O_p[p][h * DH:(h + 1) * DH, qsl], in_=ob)
                    pending[0] = pend

                for p in range(NPAIRS):
                    for t in range(NQT):
                        att_tile(p, t)
                if pending[0] is not None:
                    pending[0]()
                    pending[0] = None
                emit_outproj(3)

    return nc


def make_masks():
    import ml_dtypes

    j = np.arange(KC)[:, None]
    i = np.arange(QT)[None, :]
    return np.stack(
        [(i >= j + 128 * m) for m in range(4)]).astype(ml_dtypes.bfloat16)


def make_in_maps(x, ln_gamma, ln_beta, w_qkv, w_out):
    import ml_dtypes

    bf = ml_dtypes.bfloat16
    x = np.asarray(x, np.float32)
    g_ = np.asarray(ln_gamma, np.float32)
    b_ = np.asarray(ln_beta, np.float32)
    w_qkv = np.asarray(w_qkv, np.float32)
    w_out = np.asarray(w_out, np.float32)
    masks = make_masks()
    in_maps = []
    for c in range(8):
        b = c // 2
        g = c % 2
        cs = slice(g * INNER_PC, (g + 1) * INNER_PC)
        Wraw = np.concatenate(
            [w_qkv[:, 0 * DIM:1 * DIM][:, cs],
             w_qkv[:, 1 * DIM:2 * DIM][:, cs],
             w_qkv[:, 2 * DIM:3 * DIM][:, cs]], axis=1)
        Wp = (Wraw * g_[:, None]).astype(np.float32)
        seed = np.stack([b_ @ Wraw, Wp.sum(axis=0)]).astype(np.float32)
        in_maps.append({
            "xT": np.ascontiguousarray(x[b].T).astype(bf),
            "w": Wp.astype(bf),
            "seed": seed,
            "wo": np.ascontiguousarray(w_out[cs, :]).astype(bf),
            "masks": masks,
        })
    return in_maps


_PROG = None


def kernel(x, ln_gamma, ln_beta, w_qkv, w_out):
    global _PROG
    from concourse.bass_utils import run_bass_kernel_spmd

    if _PROG is None:
        _PROG = build_program()
    in_maps = make_in_maps(x, ln_gamma, ln_beta, w_qkv, w_out)
    res = run_bass_kernel_spmd(_PROG, in_maps, list(range(8)))
    parts = [res.results[c]["out"] for c in range(8)]
    out = np.empty((B, TOK, DIM), np.float32)
    for b in range(B):
        out[b] = parts[2 * b] + parts[2 * b + 1]
    return out


# revision 49
# speedup vs baseline: 1.2227x; 1.2227x over previous
"""Causal attention block (LN -> QKV -> causal MHA -> out-proj) on 8 trn2
NeuronCores via Bass/Tile.

Sharding: core c handles batch b=c//2 and head-group g=c%2 (8 of 16 heads).
Data parallel over batch, tensor parallel over heads; the out-proj partial
sums (2 per batch) are reduced on the host during the gather, so the device
program needs no collectives and is pure SPMD.

v2 design (vs 613us baseline):
  - All big matmuls run on bf16 operands (fp32 PSUM accumulation): halves
    SBUF footprint, DMA bytes and LDWEIGHTS traffic.
  - The PE is kept continuously busy so the HAM activity monitor never
    re-throttles the 2.4 GHz clock: LN stats matmuls overlap the x DMA;
    the rstd-independent QKV main matmuls (x is kept UNSCALED) cover the
    LN postprocessing; rstd is folded into the PSUM->SBUF evacuation of
    Q^T/K^T (scalar_tensor_tensor with a PE-broadcast rstd tile) and into
    V's ACT-engine copy (per-partition scale operand).
  - Causal diagonal trimming for query tiles t>=1: the 4 diagonal key
    chunks only compute/apply exp+mask+PV on columns [m*128:512]; masking
    shrinks to one [128,128] triangle multiply per (chunk, head).
  - Softmax denominators (the free 65th ones-column of V) are inverted
    with reciprocal_approx_fast (~0.9us) instead of the 8-cycle/element
    DVE reciprocal (3.6us each / 17us for the LN row).
  - o^T stays in SBUF (no DRAM scratch roundtrip); PSUM accumulator banks
    are evacuated immediately after the last PV so the normalization chain
    never blocks the next tile; normalization output is DMA'd SBUF->SBUF
    into the per-pair o^T tile (the DMA performs the partition shift).
  - The out-projection is interleaved with pair 3's attention tiles and
    reads o^T straight from SBUF.
"""

import numpy as np

import concourse.bass as bass
import concourse.mybir as mybir
import concourse.tile as tile_mod

# ----------------------------------------------------------------------------
# Workaround for this walrus build rejecting instructions that carry more than
# MAX_WAITS semaphore waits ("Too many sync wait commands" in CoreV3GenImpl
# setupSyncWait). Split excess waits onto single-wait NOP carrier instructions
# emitted just before the original instruction on the same engine: program
# order on the sequencer makes this semantically identical.
# ----------------------------------------------------------------------------
_MAX_WAITS = 1
_orig_add_instruction = tile_mod.TileContext._add_instruction
_carrier_id = [0]


def _split_waits_add_instruction(self, inst):
    si = inst.sync_info
    if (
        si is not None
        and si.on_wait
        and len(si.on_wait) > _MAX_WAITS
        and inst.engine != mybir.EngineType.Unassigned
    ):
        waits = list(si.on_wait)
        keep = waits[-_MAX_WAITS:]
        for w in waits[:-_MAX_WAITS]:
            _carrier_id[0] += 1
            nop = mybir.InstNoOp(name=f"I-waitc-{_carrier_id[0]}")
            nop.engine = inst.engine
            nop.sync_info = mybir.SyncInfo(on_wait=[w], on_update=[])
            _orig_add_instruction(self, nop)
        inst.sync_info = mybir.SyncInfo(
            on_wait=keep,
            on_update=list(si.on_update) if si.on_update else [],
        )
    _orig_add_instruction(self, inst)


tile_mod.TileContext._add_instruction = _split_waits_add_instruction

from concourse.vector_clock import ScopedClock


def _patched_drain_and_barrier(self, tick_clock, wait_clock):
    nc = self.nc
    carrier = nc.sync.nop(nofuse=True)
    wait_clock.add_sem_waits(carrier.ins, ScopedClock({None: tick_clock.global_clock}))
    si = carrier.ins.sync_info
    waits = list(si.on_wait) if si is not None and si.on_wait else []
    if len(waits) > _MAX_WAITS:
        carrier.ins.sync_info = mybir.SyncInfo(
            on_wait=waits[:_MAX_WAITS],
            on_update=list(si.on_update) if si.on_update else [],
        )
        rest = waits[_MAX_WAITS:]
        while rest:
            extra = nc.sync.nop(nofuse=True)
            extra.ins.sync_info = mybir.SyncInfo(
                on_wait=rest[:_MAX_WAITS], on_update=[])
            rest = rest[_MAX_WAITS:]

    nc.sync.drain()
    nc.all_engine_barrier()
    assert self.sems is not None
    popped = nc._tile_sem_poison_stack.pop()
    assert popped is self._sem_poison
    nc.clear_and_free_semaphores(list(self.sems.allocated().values()))
    nc.all_engine_barrier()


tile_mod.TileContext._drain_and_barrier = _patched_drain_and_barrier

# ----------------------------------------------------------------------------

F32 = mybir.dt.float32
F32R = mybir.dt.float32r
BF16 = mybir.dt.bfloat16
ALU = mybir.AluOpType
ACT_EXP = mybir.ActivationFunctionType.Exp
ACT_LN = mybir.ActivationFunctionType.Ln

U32 = mybir.dt.uint32
ONE_BITS = int(np.float32(1.0).view(np.uint32))

B = 4
TOK = 2048
DIM = 1024
HEADS = 16
DH = 64
HEADS_PC = 8          # heads per core
INNER_PC = HEADS_PC * DH  # 512
NPAIRS = HEADS_PC // 2
QT = 512              # query tile
KC = 128              # key-token chunk
NKD = DIM // 128      # 8 contraction chunks
NQT = TOK // QT       # 4 query tiles
NTT = TOK // KC       # 16 token chunks
EPS = 1e-5
SCALE = DH ** -0.5


def _r(ap):
    """View an fp32 AP as float32r for full-rate PE matmuls."""
    return ap.bitcast(F32R)


def build_program():
    nc = bass.Bass()
    xT = nc.declare_dram_parameter("xT", [DIM, TOK], BF16, isOutput=False)
    w = nc.declare_dram_parameter("w", [DIM, 3 * INNER_PC], BF16, isOutput=False)
    seed = nc.declare_dram_parameter("seed", [2, 3 * INNER_PC], F32R, isOutput=False)
    wo = nc.declare_dram_parameter("wo", [INNER_PC, DIM], BF16, isOutput=False)
    masks = nc.declare_dram_parameter("masks", [4, KC, QT], BF16, isOutput=False)
    out = nc.declare_dram_parameter("out", [TOK, DIM], F32, isOutput=True)
    rstd_dram = nc.dram_tensor("rstd_scr", [1, TOK], F32R)

    w4 = w.rearrange("(kc p) (g c) -> p kc g c", p=128, c=128)
    wo4 = wo.rearrange("(pp p) d -> p pp d", p=128)

    with tile_mod.TileContext(nc) as tc, nc.allow_low_precision(
            "bf16 matmul operands; all accumulation in fp32 PSUM"):
        with (
            tc.tile_pool(name="const", bufs=1) as const,
            tc.tile_pool(name="xtp", bufs=NKD) as xtp,
            tc.tile_pool(name="wqkp", bufs=NPAIRS) as wqkp,
            tc.tile_pool(name="wbig", bufs=1) as wbig,
            tc.tile_pool(name="vpool", bufs=NTT) as vpool,
            tc.tile_pool(name="qkp", bufs=2) as qkp,
            tc.tile_pool(name="opp", bufs=NPAIRS) as opp,
            tc.tile_pool(name="psb", bufs=3) as ppool,
            tc.tile_pool(name="orawp", bufs=4) as orawp,
            tc.tile_pool(name="osbp", bufs=4) as osbp,
            tc.tile_pool(name="sqp", bufs=2) as sqp,
            tc.tile_pool(name="recp", bufs=4) as recp,
            tc.tile_pool(name="outsb", bufs=2) as outsbp,
        ):
            # ---------------- constants / weights ----------------
            ones_col = const.tile([128, 1], BF16, tag="ones_col")
            nc.vector.memset(ones_col, 1.0)
            ones_row = const.tile([1, 128], F32R, tag="ones_row")
            nc.vector.memset(ones_row.bitcast(U32), ONE_BITS)
            ones_row_bf = const.tile([1, 128], BF16, tag="ones_row_bf")
            nc.vector.memset(ones_row_bf, 1.0)
            eps_t = const.tile([1, 1], F32, tag="eps")
            nc.vector.memset(eps_t, EPS)
            seed_sb = const.tile([2, 3 * INNER_PC], F32R, tag="seed")
            nc.sync.dma_start(out=seed_sb, in_=seed[:, :])
            mask_sb = const.tile([KC, 4, QT], BF16, tag="mask")
            for m in range(4):
                nc.scalar.dma_start(out=mask_sb[:, m, :], in_=masks[m, :, :])
            onmr2 = const.tile([2, TOK], F32R, tag="onmr2")  # row0=std, row1=-mu
            r0 = const.tile([1, TOK], F32R, tag="r0")  # sums -> -mu
            r1 = const.tile([1, TOK], F32R, tag="r1")  # sumsq -> ex2 -> var -> L
            rstd_row = const.tile([1, TOK], F32R, tag="rstd_row")  # musq -> rstd
            bc_sb = const.tile([128, TOK], F32, tag="bc")    # rstd col-broadcast
            rstd_col = const.tile([128, NTT], F32R, tag="rstd_col")

            # ---------------- phase A: x load + LN stats ----------------
            xt = []
            with tc.tile_pool(name="pstat", bufs=1, space="PSUM") as pstat:
                sum_ps = pstat.tile([1, TOK], F32, tag="sum")
                sq_ps = pstat.tile([1, TOK], F32, tag="sq")
                for kc in range(NKD):
                    t_ = xtp.tile([128, TOK], BF16, tag="xt", name=f"xt{kc}")
                    nc.sync.dma_start(out=t_, in_=xT[kc * 128:(kc + 1) * 128, :])
                    xt.append(t_)
                    sq_t = sqp.tile([128, TOK], BF16, tag="sq_t")
                    nc.vector.tensor_mul(sq_t, t_, t_)
                    for s in range(NQT):
                        sl = slice(s * QT, (s + 1) * QT)
                        nc.tensor.matmul(
                            out=sum_ps[0:1, sl], lhsT=ones_col, rhs=t_[:, sl],
                            start=(kc == 0), stop=(kc == NKD - 1))
                        nc.tensor.matmul(
                            out=sq_ps[0:1, sl], lhsT=ones_col, rhs=sq_t[:, sl],
                            start=(kc == 0), stop=(kc == NKD - 1))
                # evacuate stats (ACT + DVE in parallel) so pstat banks free
                nc.scalar.copy(r0, sum_ps)
                nc.vector.tensor_copy(r1, sq_ps)

            wqk = []
            for p in range(NPAIRS):
                t_ = wqkp.tile([128, NKD, 256], BF16, tag="wqk", name=f"wqk{p}")
                eng = nc.scalar if p == 0 else nc.sync
                for kc in range(NKD):
                    eng.dma_start(out=t_[:, kc, 0:128], in_=w4[:, kc, p, :])
                    eng.dma_start(out=t_[:, kc, 128:256],
                                  in_=w4[:, kc, 4 + p, :])
                wqk.append(t_)
            wv = wbig.tile([128, NKD, INNER_PC], BF16, tag="wv")
            for kc in range(NKD):
                nc.scalar.dma_start(
                    out=wv[:, kc, :].rearrange("p (g c) -> p g c", c=128),
                    in_=w4[:, kc, 8:12, :])
            wo_sb = wbig.tile([128, NPAIRS, DIM], BF16, tag="wo")
            for p in range(NPAIRS):
                nc.sync.dma_start(out=wo_sb[:, p, :], in_=wo4[:, p, :])


            with (
                tc.tile_pool(name="pss", bufs=2, space="PSUM") as pss,
                tc.tile_pool(name="pso", bufs=2, space="PSUM") as pso,
                tc.tile_pool(name="scr", bufs=2, space="PSUM") as scr,
            ):
                # ------------- LN postprocessing (DVE/ACT only) -------------
                nc.vector.tensor_scalar_mul(r0, r0, -1.0 / DIM)          # -mu
                nc.scalar.dma_start(out=onmr2[1:2, :], in_=r0)           # -mu row
                nc.vector.tensor_scalar_mul(r1, r1, 1.0 / DIM)           # ex2
                nc.scalar.square(rstd_row, r0)                           # mu^2
                nc.vector.tensor_sub(r1, r1, rstd_row)                   # var
                nc.scalar.activation(out=r1, in_=r1,
                                     func=ACT_LN, bias=eps_t, scale=1.0)
                nc.scalar.activation(out=rstd_row, in_=r1,
                                     func=ACT_EXP, scale=-0.5)           # rstd
                nc.scalar.activation(out=onmr2[0:1, :], in_=r1,
                                     func=ACT_EXP, scale=0.5)            # std
                # rstd transposed to columns via a DRAM roundtrip (for V's
                # per-partition ACT scale)
                nc.scalar.dma_start(out=rstd_dram[:, :], in_=rstd_row)
                nc.scalar.dma_start(
                    out=rstd_col,
                    in_=rstd_dram.rearrange("o (i p) -> p (o i)", p=128))

                qT = {}
                kT = {}

                def emit_qk(p, dst, t, pool, defer_stt=False):
                    """Project 128 cols of Q^T (dst=0) or K^T (dst=1) for
                    tokens of tile t; fold rstd via STT on evacuation."""
                    if dst == 0 and p not in qT:
                        qT[p] = qkp.tile([128, TOK], BF16, tag="qT",
                                         name=f"qT{p}")
                    if dst == 1 and p not in kT:
                        kT[p] = qkp.tile([128, TOK], BF16, tag="kT",
                                         name=f"kT{p}")
                    dstT = qT[p] if dst == 0 else kT[p]
                    sl = slice(t * QT, (t + 1) * QT)
                    tag = "s_ps" if pool is pss else ("o_ps" if pool is pso else "scr")
                    ps = pool.tile([128, QT], F32, tag=tag, name=f"qk{p}{dst}{t}")
                    for kc in range(NKD):
                        nc.tensor.matmul(
                            out=ps, lhsT=wqk[p][:, kc, dst * 128:dst * 128 + 128],
                            rhs=xt[kc][:, sl], start=(kc == 0), stop=False)
                    nc.tensor.matmul(
                        out=ps,
                        lhsT=seed_sb[:, dst * INNER_PC + p * 128:
                                     dst * INNER_PC + (p + 1) * 128],
                        rhs=onmr2[:, sl], start=False, stop=True)

                    def stt():
                        nc.vector.scalar_tensor_tensor(
                            out=dstT[:, sl], in0=ps, scalar=1.0,
                            in1=bc_sb[:, sl], op0=ALU.mult, op1=ALU.mult)
                    if defer_stt:
                        return stt
                    stt()

                v_sb = {}

                def emit_v(tt, pool):
                    """V (token-major) for token chunk tt + ones column."""
                    tsl = slice(tt * KC, (tt + 1) * KC)
                    tag = "s_ps" if pool is pss else ("o_ps" if pool is pso else "scr")
                    ps = pool.tile([128, INNER_PC], F32, tag=tag, name=f"v{tt}")
                    for kc in range(NKD):
                        nc.tensor.matmul(
                            out=ps, lhsT=xt[kc][:, tsl], rhs=wv[:, kc, :],
                            start=(kc == 0), stop=False)
                    nc.tensor.matmul(
                        out=ps, lhsT=onmr2[:, tsl],
                        rhs=seed_sb[:, 2 * INNER_PC:3 * INNER_PC],
                        start=False, stop=True)
                    vt = vpool.tile([128, HEADS_PC, DH + 1], BF16, tag="v_sb",
                                    name=f"vsb{tt}")
                    nc.scalar.mul(vt[:, :, 0:DH],
                                  ps.rearrange("p (h d) -> p h d", d=DH),
                                  rstd_col.bitcast(F32)[:, tt:tt + 1])
                    nc.vector.memset(vt[:, :, DH:DH + 1], 1.0)
                    v_sb[tt] = vt

                # prologue PE work: QK(0) tile0 + V0/V1 in pss/pso slots,
                # then rstd transpose + V2/V3 + bc in scratch.
                stt_q = emit_qk(0, 0, 0, pss, defer_stt=True)
                stt_k = emit_qk(0, 1, 0, pss, defer_stt=True)
                emit_v(0, pso)
                emit_v(1, pso)
                emit_v(2, scr)
                emit_v(3, scr)
                for s in range(NQT):
                    sl = slice(s * QT, (s + 1) * QT)
                    bc_ps = scr.tile([128, QT], F32, tag="scr", name=f"bc{s}")
                    nc.tensor.matmul(out=bc_ps, lhsT=ones_row,
                                     rhs=rstd_row[:, sl], start=True, stop=True)
                    nc.vector.tensor_copy(bc_sb[:, sl], bc_ps)
                stt_q()
                stt_k()

                O_p = [opp.tile([128, TOK], BF16, tag="O_p", name=f"Op{p}")
                       for p in range(NPAIRS)]

                def emit_outproj(t):
                    for tt in range(4 * t, 4 * t + 4):
                        tsl = slice(tt * KC, (tt + 1) * KC)
                        for nb in range(DIM // QT):
                            nsl = slice(nb * QT, (nb + 1) * QT)
                            ps = scr.tile([128, QT], F32, tag="scr",
                                          name=f"out{tt}{nb}")
                            for p_ in range(NPAIRS):
                                nc.tensor.matmul(
                                    out=ps, lhsT=O_p[p_][:, tsl],
                                    rhs=wo_sb[:, p_, nsl],
                                    start=(p_ == 0), stop=(p_ == NPAIRS - 1))
                            ob = outsbp.tile([128, QT], F32, tag="out_sb")
                            nc.scalar.copy(ob, ps)
                            nc.sync.dma_start(out=out[tsl, nsl], in_=ob)

                # ---------------- attention pair loop ----------------
                pending = [None]

                def att_tile(p, t):
                    q_, k_ = qT[p], kT[p]
                    if t == 0:
                        cl = [(c, 0) for c in range(4)]
                    else:
                        cl = [(4 * t + m, m * 128) for m in range(4)] + \
                             [(c, 0) for c in range(4 * t)]
                    n = len(cl)
                    o_ps = [pso.tile([DH + 1, QT], F32, tag="o_ps",
                                     name=f"o{p}{t}{h}") for h in range(2)]
                    s_tiles = {}

                    def emit_S(i):
                        kt, lo = cl[i]
                        st = pss.tile([128, 2 * QT], F32, tag="s_ps",
                                      name=f"s{p}{t}{i}")
                        csl = slice(kt * KC, (kt + 1) * KC)
                        qsl = slice(t * QT + lo, (t + 1) * QT)
                        for h in range(2):
                            nc.tensor.matmul(
                                out=st[:, h * QT + lo:(h + 1) * QT],
                                lhsT=k_[h * DH:(h + 1) * DH, csl],
                                rhs=q_[h * DH:(h + 1) * DH, qsl],
                                start=True, stop=True)
                        s_tiles[i] = st

                    emit_S(0)
                    if n > 1:
                        emit_S(1)
                    if pending[0] is not None:
                        pending[0]()
                        pending[0] = None
                    for i, (kt, lo) in enumerate(cl):
                        st = s_tiles.pop(i)
                        pt = ppool.tile([128, 2 * QT], BF16, tag="p_sb",
                                        name=f"pt{p}{t}{i}")
                        if lo == 0:
                            nc.scalar.activation(out=pt, in_=st,
                                                 func=ACT_EXP, scale=SCALE)
                        else:
                            for h in range(2):
                                hsl = slice(h * QT + lo, (h + 1) * QT)
                                nc.scalar.activation(out=pt[:, hsl],
                                                     in_=st[:, hsl],
                                                     func=ACT_EXP, scale=SCALE)
                        if t == 0:
                            for h in range(2):
                                nc.vector.tensor_mul(
                                    pt[:, h * QT:(h + 1) * QT],
                                    pt[:, h * QT:(h + 1) * QT],
                                    mask_sb[:, i, :])
                        elif i < 4:
                            m = i
                            for h in range(2):
                                bsl = slice(h * QT + m * 128,
                                            h * QT + (m + 1) * 128)
                                nc.vector.tensor_mul(
                                    pt[:, bsl], pt[:, bsl],
                                    mask_sb[:, 0, 0:128])
                        for h in range(2):
                            nc.tensor.matmul(
                                out=o_ps[h][:, lo:QT],
                                lhsT=v_sb[kt][:, 2 * p + h, :],
                                rhs=pt[:, h * QT + lo:(h + 1) * QT],
                                start=(i == 0), stop=(i == n - 1))
                        # PE filler hooks: next QK tiles / V prefetch / outproj
                        if i == 0:
                            if p == 0 and t < 3:
                                emit_v(4 * (t + 1), scr)
                                emit_v(4 * (t + 1) + 1, scr)
                            if t < 3:
                                emit_qk(p, 0, t + 1, scr)
                            elif p < 3:
                                emit_qk(p + 1, 0, 0, scr)
                        elif i == 1:
                            if p == 0 and t < 3:
                                emit_v(4 * (t + 1) + 2, scr)
                                emit_v(4 * (t + 1) + 3, scr)
                            if t < 3:
                                emit_qk(p, 1, t + 1, scr)
                            elif p < 3:
                                emit_qk(p + 1, 1, 0, scr)
                        elif i == 2 and p == 3 and t >= 1:
                            emit_outproj(t - 1)
                        if i + 2 < n:
                            emit_S(i + 2)

                    # tile end: invert denominators + evacuate PSUM fast
                    rec = [recp.tile([1, QT], F32R, tag="rec",
                                     name=f"rec{p}{t}{h}") for h in range(2)]
                    oraw = []
                    for h in range(2):
                        nc.vector.reciprocal(rec[h], o_ps[h][DH:DH + 1, :])
                        orw = orawp.tile([DH, QT], BF16, tag="oraw",
                                         name=f"oraw{p}{t}{h}")
                        nc.vector.tensor_copy(orw, o_ps[h][0:DH, :])
                        oraw.append(orw)

                    def pend(p=p, t=t, rec=rec, oraw=oraw):
                        qsl = slice(t * QT, (t + 1) * QT)
                        for h in range(2):
                            rb = scr.tile([DH, QT], F32, tag="scr",
                                          name=f"rb{p}{t}{h}")
                            nc.tensor.matmul(
                                out=rb, lhsT=ones_row[:, 0:DH],
                                rhs=rec[h], start=True, stop=True)
                            ob = osbp.tile([DH, QT], BF16, tag="osb",
                                           name=f"osb{p}{t}{h}")
                            nc.vector.scalar_tensor_tensor(
                                out=ob, in0=oraw[h], scalar=1.0, in1=rb,
                                op0=ALU.mult, op1=ALU.mult)
                            nc.sync.dma_start(
                                out=O_p[p][h * DH:(h + 1) * DH, qsl], in_=ob)
                    pending[0] = pend

                for p in range(NPAIRS):
                    for t in range(NQT):
                        att_tile(p, t)
                if pending[0] is not None:
                    pending[0]()
                    pending[0] = None
                emit_outproj(3)

    return nc


def make_masks():
    import ml_dtypes

    j = np.arange(KC)[:, None]
    i = np.arange(QT)[None, :]
    return np.stack(
        [(i >= j + 128 * m) for m in range(4)]).astype(ml_dtypes.bfloat16)


def make_in_maps(x, ln_gamma, ln_beta, w_qkv, w_out):
    import ml_dtypes

    bf = ml_dtypes.bfloat16
    x = np.asarray(x, np.float32)
    g_ = np.asarray(ln_gamma, np.float32)
    b_ = np.asarray(ln_beta, np.float32)
    w_qkv = np.asarray(w_qkv, np.float32)
    w_out = np.asarray(w_out, np.float32)
    masks = make_masks()
    in_maps = []
    for c in range(8):
        b = c // 2
        g = c % 2
        cs = slice(g * INNER_PC, (g + 1) * INNER_PC)
        Wraw = np.concatenate(
            [w_qkv[:, 0 * DIM:1 * DIM][:, cs],
             w_qkv[:, 1 * DIM:2 * DIM][:, cs],
             w_qkv[:, 2 * DIM:3 * DIM][:, cs]], axis=1)
        Wp = (Wraw * g_[:, None]).astype(np.float32)
        seed = np.stack([b_ @ Wraw, Wp.sum(axis=0)]).astype(np.float32)
        in_maps.append({
            "xT": np.ascontiguousarray(x[b].T).astype(bf),
            "w": Wp.astype(bf),
            "seed": seed,
            "wo": np.ascontiguousarray(w_out[cs, :]).astype(bf),
            "masks": masks,
        })
    return in_maps


_PROG = None


def kernel(x, ln_gamma, ln_beta, w_qkv, w_out):
    global _PROG
    from concourse.bass_utils import run_bass_kernel_spmd

    if _PROG is None:
        _PROG = build_program()
    in_maps = make_in_maps(x, ln_gamma, ln_beta, w_qkv, w_out)
    res = run_bass_kernel_spmd(_PROG, in_maps, list(range(8)))
    parts = [res.results[c]["out"] for c in range(8)]
    out = np.empty((B, TOK, DIM), np.float32)
    for b in range(B):
        out[b] = parts[2 * b] + parts[2 * b + 1]
    return out
